# revision 10
# baseline (speedup 1.0000x reference)
"""Shapelet distance transform kernel for Trainium2 (8 NeuronCores).

out[b, s] = min_w sum_{l,c} (data[b, w+l, c] - kernel[s, l, c])^2 / LS

Strategy (data-parallel over batch, 4 batches per core, no collectives),
v4 "even/odd scan": PE emits per-window distances via one matmul per
512 windows (kaug rows = -2*K/LS for the 96 im2col rows plus 32 tap
rows of 1/LS that turn staged d2 values into the sliding norm a2/LS),
split so EVEN windows land in one PSUM tile and ODD windows in another.
The drain is then:

  ACT:  odd dists  PSUM -> SBUF bf16 copy      (ACT's own PSUM port)
  DVE:  tensor_tensor_scan(op0=min, op1=min):
          state = min(min(ev[t], state), od[t])   -- 2 windows/step
        chained across groups via initial=prev_out[:, -1:]

so each engine touches each window's dist exactly once and the DVE
does 0.5 cycles/window instead of the baseline's 1 (the baseline DVE
tensor_reduce from PSUM was the 68us/core floor; tensor_tensor_reduce
would fuse better still but wedges the device on this toolchain).

The matmul rhs tiles need no on-chip reformatting: phase A stages the
data (and d2) to DRAM in a parity-split layout xs[g][par][u] =
stream_g[2u+par] (g = 3 channels + d2 taps), so an rhs tile
[128 rows = (par, g, lag-half), 1024 windows of one parity] is two
affine DMAs (one per parity, contiguous partitions). Window coverage:
groups of 2048 windows at starts {0, 2048, 4096, 6113}; the odd-start
last group exactly covers [0, 8161) with a benign 31-window repeat.

Engine budget per core (4 batches): DVE 32 scans of 1024 ~ 36us, ACT
32 copies of 1024 ~ 30us, PE 128 matmuls of 512 bf16 (self-loading
weights) ~ 35us, DMA ~ 9MB spread over 16 queues.
"""

import sys

for _p in ("/opt/trn_rl_repo",):
    if _p not in sys.path:
        sys.path.insert(0, _p)

from contextlib import ExitStack

import ml_dtypes
import numpy as np

import concourse.bacc as bacc
import concourse.bass as bass
import concourse.tile as tile
from concourse import mybir

F32 = mybir.dt.float32
BF16 = mybir.dt.bfloat16
AL = mybir.AluOpType
AF = mybir.ActivationFunctionType

B, T, C = 32, 8192, 3
NS, LS = 256, 32
W = T - LS + 1  # 8161 valid windows
NCORES = 8
BL = B // NCORES  # 4 batches per core
FLAT = T * C  # 24576
FLATP = FLAT + 192  # host zero-pads so the 66-t overlap load is in bounds
SCALE = 1.0 / LS
GROUPS = [0, 2048, 4096, 6113]  # window starts; 2048 windows each
SECT = 8192  # stream section per g (2 parities x 4096)
SBATCH = 4 * SECT  # xs elements per batch


def build_program() -> bass.Bass:
    nc = bacc.Bacc("TRN2", target_bir_lowering=False, debug=False)
    data = nc.dram_tensor("data", [BL, FLATP], BF16, kind="ExternalInput").ap()
    kaug = nc.dram_tensor("kaug", [2, 128, 128], BF16, kind="ExternalInput").ap()
    k2c = nc.dram_tensor("k2c", [2, 128], F32, kind="ExternalInput").ap()
    out = nc.dram_tensor("out", [BL, NS], F32, kind="ExternalOutput").ap()
    xs = nc.dram_tensor("xs", [BL, SBATCH], BF16).ap()

    with tile.TileContext(nc) as tc, ExitStack() as ctx:
        consts = ctx.enter_context(tc.tile_pool(name="consts", bufs=1))
        kaug_sb = consts.tile([128, 2, 128], BF16)
        nc.sync.dma_start(
            out=kaug_sb,
            in_=bass.AP(
                tensor=kaug.tensor,
                offset=kaug.offset,
                ap=[[128, 128], [128 * 128, 2], [1, 128]],
            ),
        )
        k2sb = consts.tile([128, 2], F32)
        nc.sync.dma_start(
            out=k2sb,
            in_=bass.AP(
                tensor=k2c.tensor, offset=k2c.offset, ap=[[1, 128], [128, 2]]
            ),
        )

        # ---- Phase A: parity-split streams staged to DRAM. ----
        pA = ctx.enter_context(tc.tile_pool(name="pA", bufs=2))
        for b in range(BL):
            F = pA.tile([128, 198], BF16, tag="F")
            nc.sync.dma_start(
                out=F,
                in_=bass.AP(
                    tensor=data.tensor,
                    offset=data.offset + b * FLATP,
                    ap=[[192, 128], [1, 198]],
                ),
            )

            # px[p, c, par, v] = x3[c, 64p + 2v + par] = F[p, 3*(2v+par) + c]
            px = pA.tile([128, 3, 2, 32], BF16, tag="px")
            for par in range(2):
                nc.scalar.copy(
                    px[:, :, par, :],
                    bass.AP(
                        tensor=F.tensor,
                        offset=F.offset + 3 * par,
                        ap=[F.ap[0], [1, 3], [6, 32]],
                    ),
                )

            # d2 path: d2loc[p, v] = sum_c F[p, 3v+c]^2, v in [0, 66)
            sq = pA.tile([128, 198], F32, tag="sq")
            nc.scalar.square(sq, F)
            d2loc = pA.tile([128, 66], F32, tag="d2loc")
            nc.vector.tensor_reduce(
                d2loc,
                bass.AP(tensor=sq.tensor, offset=sq.offset, ap=[sq.ap[0], [3, 66], [1, 3]]),
                axis=mybir.AxisListType.X,
                op=AL.add,
            )
            pd = pA.tile([128, 2, 32], BF16, tag="pd")
            for par in range(2):
                nc.vector.tensor_copy(
                    pd[:, par, :],
                    bass.AP(
                        tensor=d2loc.tensor,
                        offset=d2loc.offset + par,
                        ap=[d2loc.ap[0], [2, 32]],
                    ),
                )

            # stores (DMA APs max 3 dims: one store per parity for x)
            for par in range(2):
                nc.sync.dma_start(
                    out=bass.AP(
                        tensor=xs.tensor,
                        offset=xs.offset + b * SBATCH + par * 4096,
                        ap=[[32, 128], [SECT, 3], [1, 32]],
                    ),
                    in_=px[:, :, par, :],
                )
            nc.sync.dma_start(
                out=bass.AP(
                    tensor=xs.tensor,
                    offset=xs.offset + b * SBATCH + 3 * SECT,
                    ap=[[32, 128], [4096, 2], [1, 32]],
                ),
                in_=pd,
            )

        # ---- Phase B: matmuls + even/odd min scan. ----
        rhs_pool = ctx.enter_context(tc.tile_pool(name="rhs", bufs=2))
        ps_ev = ctx.enter_context(tc.tile_pool(name="ps_ev", bufs=2, space="PSUM"))
        ps_od = ctx.enter_context(tc.tile_pool(name="ps_od", bufs=2, space="PSUM"))
        odc_pool = ctx.enter_context(tc.tile_pool(name="odc", bufs=3))
        scr_pool = ctx.enter_context(tc.tile_pool(name="scr", bufs=3))
        fin_pool = ctx.enter_context(tc.tile_pool(name="fin", bufs=4))

        def load_windows(tile_, b, w0, n=1024):
            # rhs rows p = par*64 + g*16 + lh hold stream_g[w0 + 2i + l],
            # l = 2*lh + par, i in [0, n): parity-split layout makes this
            # one u-contiguous DMA per parity (contiguous partitions).
            for par in range(2):
                if w0 % 2 == 0:
                    base = b * SBATCH + par * 4096 + w0 // 2
                else:
                    base = b * SBATCH + (1 - par) * 4096 + (w0 + par) // 2
                nc.sync.dma_start(
                    out=tile_[par * 64 : (par + 1) * 64, :],
                    in_=bass.AP(
                        tensor=xs.tensor,
                        offset=xs.offset + base,
                        ap=[[SECT, 4], [1, 16], [1, n]],
                    ),
                )

        for b in range(BL):
            evt = []
            odt = []
            for gi, w0 in enumerate(GROUPS):
                et = rhs_pool.tile([128, 1024], BF16, tag=f"ev{gi}")
                load_windows(et, b, w0)
                ot = rhs_pool.tile([128, 1024], BF16, tag=f"od{gi}")
                load_windows(ot, b, w0 + 1)
                evt.append(et)
                odt.append(ot)
            for sc in range(2):
                scr_prev = None
                for gi in range(4):
                    pse = ps_ev.tile([128, 1024], F32)
                    pso = ps_od.tile([128, 1024], F32)
                    for h in range(2):
                        hs = slice(h * 512, (h + 1) * 512)
                        nc.tensor.matmul(
                            pse[:, hs], kaug_sb[:, sc, :], evt[gi][:, hs],
                            start=True, stop=True,
                        )
                        nc.tensor.matmul(
                            pso[:, hs], kaug_sb[:, sc, :], odt[gi][:, hs],
                            start=True, stop=True,
                        )
                    odc = odc_pool.tile([128, 1024], BF16, tag="o")
                    nc.scalar.copy(odc, pso)
                    scr = scr_pool.tile([128, 1024], F32, tag="s")
                    nc.vector.tensor_tensor_scan(
                        out=scr,
                        data0=pse,
                        data1=odc,
                        initial=(1e30 if scr_prev is None else scr_prev[:, 1023:1024]),
                        op0=AL.min,
                        op1=AL.min,
                    )
                    scr_prev = scr
                fin = fin_pool.tile([128, 1], F32, tag="fin")
                nc.vector.tensor_scalar(
                    out=fin,
                    in0=scr_prev[:, 1023:1024],
                    scalar1=k2sb[:, sc : sc + 1],
                    scalar2=None,
                    op0=AL.add,
                )
                nc.sync.dma_start(
                    out=out[b, sc * 128 : (sc + 1) * 128].rearrange("(p o) -> p o", o=1),
                    in_=fin,
                )
    nc.compile()
    return nc


_PROGRAM = None


def _get_program() -> bass.Bass:
    global _PROGRAM
    if _PROGRAM is None:
        _PROGRAM = build_program()
    return _PROGRAM


def make_in_maps(data: np.ndarray, kernel: np.ndarray) -> list[dict]:
    assert data.shape == (B, T, C) and kernel.shape == (NS, LS, C)
    flat = np.zeros((B, FLATP), dtype=ml_dtypes.bfloat16)
    flat[:, :FLAT] = np.ascontiguousarray(data, dtype=np.float32).reshape(B, FLAT)
    kb = np.ascontiguousarray(kernel, dtype=np.float32).astype(ml_dtypes.bfloat16)
    kf = kb.astype(np.float32)  # [NS, LS, C]
    kaug = np.zeros((2, 128, 128), dtype=np.float32)
    for sc in range(2):
        ks = kf[sc * 128 : (sc + 1) * 128]  # [128, LS, C]
        for c_ in range(C):
            for l in range(LS):
                row = (l % 2) * 64 + c_ * 16 + l // 2
                kaug[sc, row, :] = -2.0 * SCALE * ks[:, l, c_]
        for l in range(LS):  # d2 tap rows (g-section 3)
            kaug[sc, (l % 2) * 64 + 48 + l // 2, :] = SCALE
    kaug = kaug.astype(ml_dtypes.bfloat16)
    k2 = ((kf * kf).sum(axis=(1, 2)) * SCALE).astype(np.float32)  # [NS]
    k2c = np.stack([k2[:128], k2[128:]]).astype(np.float32)  # [2, 128]
    maps = [
        {
            "data": np.ascontiguousarray(flat[i * BL : (i + 1) * BL]),
            "kaug": kaug,
            "k2c": k2c,
        }
        for i in range(NCORES)
    ]
    return maps


def kernel(data: np.ndarray, kernel: np.ndarray) -> np.ndarray:
    from concourse.bass_utils import run_bass_kernel_spmd

    in_maps = make_in_maps(data, kernel)
    nc = _get_program()
    res = run_bass_kernel_spmd(nc, in_maps, list(range(NCORES)))
    return np.concatenate(
        [res.results[i]["out"] for i in range(NCORES)], axis=0
    ).astype(np.float32)


# revision 12
# speedup vs baseline: 1.1598x; 1.1598x over previous
"""Shapelet distance transform kernel for Trainium2 (8 NeuronCores).

out[b, s] = min_w sum_{l,c} (data[b, w+l, c] - kernel[s, l, c])^2 / LS

Strategy (data-parallel over batch, 4 batches per core, no collectives):
PE emits per-window distances directly -- kaug rows are -2*K/LS for the
96 im2col rows plus 32 tap rows of 1/LS that turn staged d2 values into
the sliding-norm a2/LS -- and the DVE min-reduces each [128, 1024] PSUM
tile straight to one column of a mins tile.

Measured op-throughput on this HW pinned the design: every DVE
min-combining form (tensor_reduce, tensor_tensor, scan) runs at ~1
elem/lane/cycle (tensor_tensor_reduce wedges the device; GpSimd cannot
run TensorTensor; no 2x DVE mode engages for min), so the drain floor
is ~1.04 ns/window on DVE and direct tensor_reduce from PSUM is
optimal. The win over the reference kernel is everything else: the
matmul rhs is built by pure DMA from a DRAM-staged parity-split layout
xs[g][par][u] = stream_g[2u+par] (g = 3 channels + d2 taps), so rhs
rows (par*64 + g*16 + lag-half) load u-contiguously -- no PE
transposes, no ACT im2col copies (baseline: 71us ACT + 24us PE
transposes + fp32r weight loads).

Window coverage: 8 groups of 1024 windows at starts {0, 1024, ...,
6144, 7137}; the odd-start last group (parity sections swap in its AP)
exactly covers [0, 8161) with a benign 31-window repeat, so no
phantom-window poisoning is needed.

Engine budget per core (4 batches): DVE 64 reduces of 1024 ~ 70us
(the floor), PE 128 matmuls of 512 bf16 ~ 40-55us, ACT only the small
phase-A ops, DMA ~ 8.5MB over 16 queues.
"""

import sys

for _p in ("/opt/trn_rl_repo",):
    if _p not in sys.path:
        sys.path.insert(0, _p)

from contextlib import ExitStack

import ml_dtypes
import numpy as np

import concourse.bacc as bacc
import concourse.bass as bass
import concourse.tile as tile
from concourse import mybir

F32 = mybir.dt.float32
BF16 = mybir.dt.bfloat16
AL = mybir.AluOpType
AF = mybir.ActivationFunctionType

B, T, C = 32, 8192, 3
NS, LS = 256, 32
W = T - LS + 1  # 8161 valid windows
NCORES = 8
BL = B // NCORES  # 4 batches per core
FLAT = T * C  # 24576
FLATP = FLAT + 192  # host zero-pads so the 66-t overlap load is in bounds
SCALE = 1.0 / LS
GROUPS = [0, 1024, 2048, 3072, 4096, 5120, 6144, 7137]  # 1024 windows each
NG = len(GROUPS)
SECT = 8192  # stream section per g (2 parities x 4096)
SBATCH = 4 * SECT  # xs elements per batch


def build_program() -> bass.Bass:
    nc = bacc.Bacc("TRN2", target_bir_lowering=False, debug=False)
    data = nc.dram_tensor("data", [BL, FLATP], BF16, kind="ExternalInput").ap()
    kaug = nc.dram_tensor("kaug", [2, 128, 128], BF16, kind="ExternalInput").ap()
    k2c = nc.dram_tensor("k2c", [2, 128], F32, kind="ExternalInput").ap()
    out = nc.dram_tensor("out", [BL, NS], F32, kind="ExternalOutput").ap()
    xs = nc.dram_tensor("xs", [BL, SBATCH], BF16).ap()

    with tile.TileContext(nc) as tc, ExitStack() as ctx:
        consts = ctx.enter_context(tc.tile_pool(name="consts", bufs=1))
        kaug_sb = consts.tile([128, 2, 128], BF16)
        nc.sync.dma_start(
            out=kaug_sb,
            in_=bass.AP(
                tensor=kaug.tensor,
                offset=kaug.offset,
                ap=[[128, 128], [128 * 128, 2], [1, 128]],
            ),
        )
        k2sb = consts.tile([128, 2], F32)
        nc.sync.dma_start(
            out=k2sb,
            in_=bass.AP(
                tensor=k2c.tensor, offset=k2c.offset, ap=[[1, 128], [128, 2]]
            ),
        )

        # ---- Phase A: parity-split streams staged to DRAM. ----
        pA = ctx.enter_context(tc.tile_pool(name="pA", bufs=2))
        for b in range(BL):
            F = pA.tile([128, 192], BF16, tag="F")
            nc.sync.dma_start(
                out=F,
                in_=bass.AP(
                    tensor=data.tensor,
                    offset=data.offset + b * FLATP,
                    ap=[[192, 128], [1, 192]],
                ),
            )

            # de-interleave: px[p, c, v] = x3[c, 64p + v] = F[p, 3v + c]
            px = pA.tile([128, 3, 64], BF16, tag="px")
            nc.scalar.copy(
                px,
                bass.AP(tensor=F.tensor, offset=F.offset, ap=[F.ap[0], [1, 3], [3, 64]]),
            )

            # d2 path: d2loc[p, v] = sum_c F[p, 3v+c]^2, v in [0, 64)
            sq = pA.tile([128, 192], F32, tag="sq")
            nc.scalar.square(sq, F)
            d2loc = pA.tile([128, 64], F32, tag="d2loc")
            nc.vector.tensor_reduce(
                d2loc,
                bass.AP(tensor=sq.tensor, offset=sq.offset, ap=[sq.ap[0], [3, 64], [1, 3]]),
                axis=mybir.AxisListType.X,
                op=AL.add,
            )
            pd = pA.tile([128, 64], BF16, tag="pd")
            nc.vector.tensor_copy(pd, d2loc)

            # stores: xs[b] = [x3 ch0 | x3 ch1 | x3 ch2 | d2], 8192 each
            nc.sync.dma_start(
                out=bass.AP(
                    tensor=xs.tensor,
                    offset=xs.offset + b * SBATCH,
                    ap=[[64, 128], [SECT, 3], [1, 64]],
                ),
                in_=px,
            )
            nc.sync.dma_start(
                out=bass.AP(
                    tensor=xs.tensor,
                    offset=xs.offset + b * SBATCH + 3 * SECT,
                    ap=[[64, 128], [1, 64]],
                ),
                in_=pd,
            )

        # ---- Phase B: matmuls + direct min-reduce drain. ----
        rhs_pool = ctx.enter_context(tc.tile_pool(name="rhs", bufs=2))
        ps_pool = ctx.enter_context(tc.tile_pool(name="ps", bufs=4, space="PSUM"))
        mins_pool = ctx.enter_context(tc.tile_pool(name="mins", bufs=4))
        fin_pool = ctx.enter_context(tc.tile_pool(name="fin", bufs=8))

        def load_windows(tile_, b, w0, n=1024):
            # rhs row p = g*32 + l holds stream_g[w0 + l + i], i in [0, n):
            # one t-contiguous DMA covers the whole tile.
            nc.sync.dma_start(
                out=tile_,
                in_=bass.AP(
                    tensor=xs.tensor,
                    offset=xs.offset + b * SBATCH + w0,
                    ap=[[SECT, 4], [1, 32], [1, n]],
                ),
            )

        for b in range(BL):
            rhs = []
            for gi, w0 in enumerate(GROUPS):
                rt = rhs_pool.tile([128, 1024], BF16, tag=f"g{gi}")
                load_windows(rt, b, w0)
                rhs.append(rt)
            mins0 = mins_pool.tile([128, NG], F32, tag="m0")
            mins1 = mins_pool.tile([128, NG], F32, tag="m1")
            mins = [mins0, mins1]
            for gi in range(NG):
                for sc in range(2):
                    ps = ps_pool.tile([128, 1024], F32)
                    for h in range(2):
                        hs = slice(h * 512, (h + 1) * 512)
                        nc.tensor.matmul(
                            ps[:, hs], kaug_sb[:, sc, :], rhs[gi][:, hs],
                            start=True, stop=True,
                        )
                    nc.vector.tensor_reduce(
                        mins[sc][:, gi : gi + 1],
                        ps,
                        axis=mybir.AxisListType.X,
                        op=AL.min,
                    )
            for sc in range(2):
                res = fin_pool.tile([128, 1], F32, tag="res")
                nc.vector.tensor_reduce(
                    res, mins[sc], axis=mybir.AxisListType.X, op=AL.min
                )
                fin = fin_pool.tile([128, 1], F32, tag="fin")
                nc.vector.tensor_scalar(
                    out=fin,
                    in0=res,
                    scalar1=k2sb[:, sc : sc + 1],
                    scalar2=None,
                    op0=AL.add,
                )
                nc.sync.dma_start(
                    out=out[b, sc * 128 : (sc + 1) * 128].rearrange("(p o) -> p o", o=1),
                    in_=fin,
                )
    nc.compile()
    return nc


_PROGRAM = None


def _get_program() -> bass.Bass:
    global _PROGRAM
    if _PROGRAM is None:
        _PROGRAM = build_program()
    return _PROGRAM


def make_in_maps(data: np.ndarray, kernel: np.ndarray) -> list[dict]:
    assert data.shape == (B, T, C) and kernel.shape == (NS, LS, C)
    flat = np.zeros((B, FLATP), dtype=ml_dtypes.bfloat16)
    flat[:, :FLAT] = np.ascontiguousarray(data, dtype=np.float32).reshape(B, FLAT)
    kb = np.ascontiguousarray(kernel, dtype=np.float32).astype(ml_dtypes.bfloat16)
    kf = kb.astype(np.float32)  # [NS, LS, C]
    kaug = np.zeros((2, 128, 128), dtype=np.float32)
    for sc in range(2):
        ks = kf[sc * 128 : (sc + 1) * 128]  # [128, LS, C]
        for c_ in range(C):
            for l in range(LS):
                kaug[sc, c_ * 32 + l, :] = -2.0 * SCALE * ks[:, l, c_]
        kaug[sc, 96:128, :] = SCALE  # d2 tap rows
    kaug = kaug.astype(ml_dtypes.bfloat16)
    k2 = ((kf * kf).sum(axis=(1, 2)) * SCALE).astype(np.float32)  # [NS]
    k2c = np.stack([k2[:128], k2[128:]]).astype(np.float32)  # [2, 128]
    maps = [
        {
            "data": np.ascontiguousarray(flat[i * BL : (i + 1) * BL]),
            "kaug": kaug,
            "k2c": k2c,
        }
        for i in range(NCORES)
    ]
    return maps


def kernel(data: np.ndarray, kernel: np.ndarray) -> np.ndarray:
    from concourse.bass_utils import run_bass_kernel_spmd

    in_maps = make_in_maps(data, kernel)
    nc = _get_program()
    res = run_bass_kernel_spmd(nc, in_maps, list(range(NCORES)))
    return np.concatenate(
        [res.results[i]["out"] for i in range(NCORES)], axis=0
    ).astype(np.float32)
